# revision 72
# baseline (speedup 1.0000x reference)
"""Trainium2 Bass kernel for nn_Attention_91293824844283.

Multi-head attention (identity rep): per-head 1x1-conv Q/K/V projections,
softmax(Q K^T / sqrt(E)) V, per-head output projection summed over heads.

Shapes: B=4, N=2048, D=512, H=8, E=64.

Sharding over 8 cores: core c -> (batch b = c//2, head-group g = c%2 of 4
heads). Each core computes the partial output sum over its 4 heads for its
batch; host adds the two partials per batch.

Device-side design (per core):
  - x^T inputs (bf16, chunk-major for single-DMA chunks), packed
    transposed weights, 1/sqrt(E) folded into Wq. V augmented with a ones
    column (slot 66) so the PV matmul also emits softmax denominators
    (M=65, K=128).
  - Scalar-engine exp is the pacer (128 ACTIVATEs x ~1.1us at 1.2GHz).
    All other work is emitted INSIDE the exp-paced sweeps as small filler
    units (<=1 matmul each) so the in-order engine queues never stall
    ACT: quarter q's normalization + output projection interleave into
    quarter q+1's sweep. PV lags S/exp by one tile and the last PV + rep
    drain carry into the next sweep, keeping quarter boundaries tight.
  - Transpose-free normalization: rep stays unnormalized; the sums row is
    scattered to [128,4] via SBUF->SBUF DMA (round trip cancels layout)
    so the DVE reciprocal runs on 128 lanes; a PE outer-product
    broadcasts 1/d to [64,512]; one DVE multiply writes pre-normalized
    bf16 rep^T into pair-packed tiles (head s at partitions s*64; the
    s=1 half is partition-shifted by DMA).
  - Output projection: repP packs the head pair on the contraction axis,
    so one K=128 matmul per pair yields both heads' contributions summed;
    the two pairs accumulate in one PSUM bank; copy + DMA out.
  - DMA: one transfer per chunk; ramp-critical loads on the sync queue,
    late weights on the scalar queue's pre-exp window.
"""

import numpy as np
import ml_dtypes
from contextlib import ExitStack

B, N, D, H, E = 4, 2048, 512, 8, 64
HPC = 4            # heads per core
N_CORES = 8
NKT = N // 128     # 16 nk tiles
VSLOT = 66         # V slot: 64 V cols + 1 ones col + 1 pad
KT = D // 128      # 4 contraction tiles for projections
QW = 512           # nq quarter width

_CACHE = {}


def _build():
    import concourse.tile as tile
    from concourse import bacc, mybir

    bf16 = mybir.dt.bfloat16
    f32 = mybir.dt.float32
    Exp = mybir.ActivationFunctionType.Exp

    nc = bacc.Bacc(
        "TRN2", target_bir_lowering=False, debug=False, num_devices=N_CORES
    )
    # chunk-major x layouts: [chunk c, 128, KT*512] so each chunk is ONE
    # contiguous 512KB DMA (serial dma_starts on the issuing engine block
    # for ~transfer time — fewer, bigger transfers keep the queue short).
    xqT = nc.dram_tensor("xqT", [4, 128, KT * 512], bf16, kind="ExternalInput").ap()
    xkT = nc.dram_tensor("xkT", [4, 128, KT * 512], bf16, kind="ExternalInput").ap()
    vT = nc.dram_tensor("vT", [4, 128, KT * 512], bf16, kind="ExternalInput").ap()
    wqT = nc.dram_tensor("wqT", [2, 128, KT * 128], bf16, kind="ExternalInput").ap()
    wkT = nc.dram_tensor("wkT", [2, 128, KT * 128], bf16, kind="ExternalInput").ap()
    wvT = nc.dram_tensor("wvT", [128, KT * HPC * E], bf16, kind="ExternalInput").ap()
    woP = nc.dram_tensor("woP", [2, 128, D], bf16, kind="ExternalInput").ap()
    # odd heads' Wo^T again at partitions 0:64 — lets the tail outproj
    # read the un-shifted rsh tile directly (skips the rsh DMA hop)
    woS = nc.dram_tensor("woS", [2, E, D], bf16, kind="ExternalInput").ap()
    outp = nc.dram_tensor("outp", [NKT, 128, D], f32, kind="ExternalOutput").ap()

    with tile.TileContext(nc) as tc, ExitStack() as ctx:
        cp = ctx.enter_context(tc.tile_pool(name="const", bufs=1))

        # --- persistent SBUF tiles (chunk-major x: tile c holds all KT
        # k-subtiles of 512 columns each) ---
        xq = [cp.tile([128, KT * 512], bf16, tag=f"xq{c}", name=f"xq{c}")
              for c in range(4)]
        xk = [cp.tile([128, KT * 512], bf16, tag=f"xk{c}", name=f"xk{c}")
              for c in range(4)]
        xv = [cp.tile([128, KT * 512], bf16, tag=f"xv{c}", name=f"xv{c}")
              for c in range(4)]
        wq = [cp.tile([128, KT * 128], bf16, tag=f"wq{p}", name=f"wq{p}")
              for p in range(2)]
        wk = [cp.tile([128, KT * 128], bf16, tag=f"wk{p}", name=f"wk{p}")
              for p in range(2)]
        wv = cp.tile([128, KT * HPC * E], bf16, tag="wv", name="wv")
        wo = [cp.tile([128, D], bf16, tag=f"wo{p}", name=f"wo{p}") for p in range(2)]
        wo2 = [cp.tile([E, D], bf16, tag=f"wo2{p}", name=f"wo2{p}")
               for p in range(2)]
        qt = [cp.tile([128, N], bf16, tag=f"qt{p}", name=f"qt{p}") for p in range(2)]
        kt = [cp.tile([128, N], bf16, tag=f"kt{p}", name=f"kt{p}") for p in range(2)]
        vaug = [cp.tile([128, HPC * VSLOT], bf16, tag=f"va{t}", name=f"va{t}")
                for t in range(NKT)]
        # pre-normalized rep^T, packed pairs: head 2p+s at partitions s*64.
        # s=0 is written directly by DVE; s=1 lands in rsh (partitions
        # 0:64) and is shifted up via SBUF->SBUF DMA (DVE lanes are
        # partition-aligned; DMA can cross partitions).
        repP = [cp.tile([128, N], bf16, tag=f"rp{p}", name=f"repP{p}")
                for p in range(2)]
        rsh = [[cp.tile([E, QW], bf16, tag=f"rsh{par}{s}", name=f"rsh{par}{s}")
                for s in range(2)] for par in range(2)]
        # drain staging (double buffered by quarter parity)
        rts = [[cp.tile([65, QW], f32, tag=f"rts{par}{s}", name=f"rts{par}{s}")
                for s in range(2)] for par in range(2)]
        rcb = [[cp.tile([1, QW], bf16, tag=f"rcb{par}{s}", name=f"rcb{par}{s}")
                for s in range(2)] for par in range(2)]
        # [1,512] DVE reciprocal is pathological (one lane, iterative op,
        # ~3.3us). Round-trip the sums row through a [128,4] layout via
        # SBUF->SBUF DMAs (scatter p-major, gather back the same way — the
        # intermediate layout cancels) so recip runs on 128 lanes (~90ns).
        dT = [[cp.tile([128, 4], f32, tag=f"dT{par}{s}", name=f"dT{par}{s}")
               for s in range(2)] for par in range(2)]
        rT = [[cp.tile([128, 4], f32, tag=f"rT{par}{s}", name=f"rT{par}{s}")
               for s in range(2)] for par in range(2)]
        rTb = [[cp.tile([128, 4], bf16, tag=f"rTb{par}{s}", name=f"rTb{par}{s}")
                for s in range(2)] for par in range(2)]
        ones64 = cp.tile([1, 64], bf16, tag="ones64")
        dmy = cp.tile([128, 1], f32, tag="dmy")
        dmyo = cp.tile([128, 1], bf16, tag="dmyo")

        # --- exp table preload: first ACTIVATE triggers ACT_TABLE_LOAD
        # (~2.7us); fire it immediately so it overlaps the input DMA.
        nc.gpsimd.memset(dmy[:], 0.0)
        nc.scalar.activation(dmyo[:], dmy[:], Exp)
        nc.gpsimd.memset(ones64[:], 1.0)
        # ones columns for vaug on the (otherwise idle) vector engine
        for t in range(NKT):
            nc.vector.memset(vaug[t][:], 1.0)
        # PE warmup during the initial DMA wait: the PE's DVFS ramp runs
        # ~1.5x slow for its first dozens of matmuls; 8 dummies mean the
        # ramp projections run closer to full speed without delaying them
        # (the first chunk DMAs take ~11us anyway).
        warm_sb = cp.tile([128, 512], bf16, tag="warm_sb")
        nc.gpsimd.memset(warm_sb[:], 0.0)
        with tc.tile_pool(name="warmps", bufs=1, space="PSUM") as wps:
            wpt = wps.tile([128, 512], f32, tag="w", name="warm_ps")
            for i in range(8):
                nc.tensor.matmul(wpt[:], warm_sb[:, 0:128], warm_sb[:],
                                 start=True, stop=True)

        # --- input DMAs, ordered to unblock the pipelined ramp:
        # pair-0 K weights + xk c0 first, then Q/V weights + c0, then the
        # remaining K/V chunks (vproj tiles follow the sweep), Q c1-3,
        # pair-1 weights, output-proj weights.
        # ALL input DMAs go on the sync queue: a dma_start occupies the
        # issuing engine for roughly the transfer time, so putting any on
        # the scalar queue would push the first exp out by ~20us.
        nc.sync.dma_start(wk[0][:], wkT[0])
        nc.sync.dma_start(xk[0][:], xkT[0])
        nc.sync.dma_start(wq[0][:], wqT[0])
        nc.sync.dma_start(xq[0][:], xqT[0])
        nc.sync.dma_start(wv[:], wvT[:])
        nc.sync.dma_start(xv[0][:], vT[0])
        nc.sync.dma_start(xk[1][:], xkT[1])
        nc.sync.dma_start(xv[1][:], vT[1])
        nc.sync.dma_start(xq[1][:], xqT[1])
        for c in range(2, 4):
            nc.sync.dma_start(xk[c][:], xkT[c])
            nc.sync.dma_start(xv[c][:], vT[c])
        nc.sync.dma_start(xq[2][:], xqT[2])
        nc.sync.dma_start(xq[3][:], xqT[3])
        # late-needed weights last: first consumed by fillers ~36us in,
        # and keeping them off the ramp window gives the critical chunk-0
        # loads full HBM bandwidth
        nc.sync.dma_start(wk[1][:], wkT[1])
        nc.sync.dma_start(wq[1][:], wqT[1])
        for p in range(2):
            nc.sync.dma_start(wo[p][:], woP[p])
            nc.sync.dma_start(wo2[p][:], woS[p])

        # --- pools. PSUM: s pair tile 2 banks x bufs=2 + rep 2x1 bank +
        # fill 2x1 = 8 banks.
        sp = ctx.enter_context(tc.tile_pool(name="spsum", bufs=2, space="PSUM"))
        rp = ctx.enter_context(tc.tile_pool(name="rpsum", bufs=1, space="PSUM"))
        fpp = ctx.enter_context(tc.tile_pool(name="fill", bufs=2, space="PSUM"))
        ptp = ctx.enter_context(tc.tile_pool(name="ptile", bufs=8))
        ostp = ctx.enter_context(tc.tile_pool(name="ostp", bufs=4))

        def proj_chunk(dst, w, x, c):
            ps = fpp.tile([128, 512], f32, tag="f", name="proj_ps")
            for k in range(KT):
                nc.tensor.matmul(
                    ps[:], w[:, k * 128:(k + 1) * 128],
                    x[c][:, k * 512:(k + 1) * 512],
                    start=(k == 0), stop=(k == KT - 1),
                )
            nc.vector.tensor_copy(dst[:, c * 512:(c + 1) * 512], ps[:])

        def vproj(t):
            ps = fpp.tile([128, HPC * E], f32, tag="f", name="vproj_ps")
            c, off = t // 4, (t % 4) * 128
            for k in range(KT):
                nc.tensor.matmul(
                    ps[:], xv[c][:, k * 512 + off:k * 512 + off + 128],
                    wv[:, k * HPC * E:(k + 1) * HPC * E],
                    start=(k == 0), stop=(k == KT - 1),
                )
            for h in range(HPC):
                nc.vector.tensor_copy(
                    vaug[t][:, h * VSLOT:h * VSLOT + E],
                    ps[:, h * E:(h + 1) * E],
                )

        def norm_part(p, q, s, tail=False):
            # quarter (p,q) head s: rts holds unnormalized rep^T [65, 512]
            # (row 64 = softmax denominators). Write pre-normalized bf16
            # rep^T into repP[p] partitions s*64:(s+1)*64.
            par = (4 * p + q) % 2
            qsl = slice(q * QW, (q + 1) * QW)
            nc.vector.reciprocal(rT[par][s][:], dT[par][s][:])
            nc.vector.tensor_copy(rTb[par][s][:], rT[par][s][:])
            nc.sync.dma_start(rcb[par][s][:], rTb[par][s][:])
            bc = fpp.tile([64, QW], f32, tag="f", name="bcast")
            nc.tensor.matmul(bc[:], ones64[:], rcb[par][s][:],
                             start=True, stop=True)
            if s == 0:
                nc.vector.tensor_mul(repP[p][0:E, qsl],
                                     rts[par][s][0:E, :], bc[:])
            else:
                nc.vector.tensor_mul(rsh[par][1][:],
                                     rts[par][s][0:E, :], bc[:])
                if not tail:
                    nc.sync.dma_start(repP[p][E:128, qsl], rsh[par][1][:])

        # Output projection: repP packs the head pair on the partition
        # axis, so ONE K=128 matmul per pair computes both heads'
        # contributions already summed; the two pairs accumulate in PSUM.
        # Emitted as two 1-matmul filler units so PE work per exp window
        # stays under the pacing budget.
        _hold = {}

        def outproj_1(t):
            tsl = slice(t * 128, (t + 1) * 128)
            ps = fpp.tile([128, D], f32, tag="f", name="ops")
            _hold[("o", t)] = ps
            nc.tensor.matmul(ps[:], repP[0][:, tsl], wo[0][:],
                             start=True, stop=False)

        def outproj_2(t):
            tsl = slice(t * 128, (t + 1) * 128)
            ps = _hold.pop(("o", t))
            nc.tensor.matmul(ps[:], repP[1][:, tsl], wo[1][:],
                             start=False, stop=True)
            ost = ostp.tile([128, D], f32, tag="ost", name="ost")
            nc.vector.tensor_copy(ost[:], ps[:])
            nc.sync.dma_start(outp[t], ost[:])

        # projection chunks as four 1-matmul units (copy rides on the last)
        def proj_unit(key, dst, w, x, c, k):
            if k == 0:
                ps = fpp.tile([128, 512], f32, tag="f", name="proj_ps")
                _hold[key] = ps
            else:
                ps = _hold[key]
            nc.tensor.matmul(
                ps[:], w[:, k * 128:(k + 1) * 128],
                x[c][:, k * 512:(k + 1) * 512],
                start=(k == 0), stop=(k == KT - 1),
            )
            if k == KT - 1:
                del _hold[key]
                nc.vector.tensor_copy(dst[:, c * 512:(c + 1) * 512], ps[:])

        def sweep(p, q, pre=None, fillers=(), carry_in=None):
            """One attention quarter: 16 x (S pair, exp, PV pair).

            pre: dict t -> list of callables emitted before tile t's S.
            fillers: list of (t, fn): fn is emitted after tile t's PV (it
            executes in engine gaps while ACT paces the sweep). Slots must
            be late enough that any DMA the fn depends on has landed —
            a premature emission stalls the whole in-order PE queue.

            PV runs one tile behind S/exp, and the final PV + rep drain
            are returned as a carry closure that the NEXT sweep emits
            after its first S/exp — PV(15) depends on exp(15), so leaving
            it before S(q+1,0) in the PE queue would stall the exp pacer
            ~1us at every quarter boundary.
            """
            fq = {}
            for slot, fn in fillers:
                fq.setdefault(slot, []).append(fn)
            qoff = q * QW
            rep = [rp.tile([65, QW], f32, tag=f"rep{s}", name=f"rep{s}")
                   for s in range(2)]

            def pv(t, pt):
                for s in range(2):
                    h = 2 * p + s
                    vsl = slice(h * VSLOT, h * VSLOT + 65)
                    nc.tensor.matmul(
                        rep[s][:],
                        vaug[t][:, vsl], pt[:, s * QW:(s + 1) * QW],
                        start=(t == 0), stop=(t == NKT - 1),
                    )

            # PV lags S/exp by TWO tiles; the previous sweep's last two
            # PVs + drain are carried in and emitted at t=0 and t=1, so no
            # boundary window ever exceeds the exp pace.
            pts = []
            for t in range(NKT):
                if pre:
                    for fn in pre.get(t, ()):
                        fn()
                tsl = slice(t * 128, (t + 1) * 128)
                spair = sp.tile([128, 2 * QW], f32, tag="s", name="spair")
                for s in range(2):
                    esl = slice(s * 64, (s + 1) * 64)
                    nc.tensor.matmul(
                        spair[:, s * QW:(s + 1) * QW],
                        kt[p][esl, tsl], qt[p][esl, qoff:qoff + QW],
                        start=True, stop=True,
                    )
                pt = ptp.tile([128, 2 * QW], bf16, tag="p", name="pt")
                nc.scalar.activation(pt[:], spair[:], Exp)
                pts.append(pt)
                if t <= 1 and carry_in is not None:
                    carry_in[t]()
                if t >= 2:
                    pv(t - 2, pts[t - 2])
                for fn in fq.get(t, ()):
                    fn()
            # flush filler units slotted past the last tile
            for slot in sorted(k for k in fq if k >= NKT):
                for fn in fq[slot]:
                    fn()

            def carry_a():
                pv(NKT - 2, pts[NKT - 2])

            def carry_b():
                pv(NKT - 1, pts[NKT - 1])
                # drain rep -> rts staging (frees the rep PSUM banks) and
                # kick off the sums-row scatter for the reciprocal chain
                par = (4 * p + q) % 2
                for s in range(2):
                    nc.vector.tensor_copy(rts[par][s][:], rep[s][:])
                    nc.sync.dma_start(dT[par][s][:], rts[par][s][64:65, :])
            return (carry_a, carry_b)

        # --- ramp: minimal work to start sweep (0,0) — only the K/Q
        # chunk-0 projections gate the first exp; vprojs ride in-sweep
        # (PV lags two tiles, so vproj(t) at pre[t+1] is still in time)
        proj_chunk(kt[0], wk[0], xk, 0)
        proj_chunk(qt[0], wq[0], xq, 0)

        # --- emission schedule ---
        pre00 = {}
        for c in range(1, 4):
            pre00.setdefault(4 * c, []).append(
                (lambda cc: lambda: proj_chunk(kt[0], wk[0], xk, cc))(c))
        for t in range(NKT - 1):
            pre00.setdefault(t + 1, []).append((lambda tt: lambda: vproj(tt))(t))
        # qproj c1's input DMA lands ~15us in — slot it late in the sweep
        # so the in-order PE queue never blocks on it.
        cr = sweep(0, 0, pre=pre00,
                   fillers=[(13, lambda: proj_chunk(qt[0], wq[0], xq, 1)),
                            (16, lambda: vproj(NKT - 1))])

        def nf(p, q, s):
            return lambda: norm_part(p, q, s)

        def pfs(dst, w, x, c, base):
            # proj chunk as four consecutive-slot 1-matmul units
            key = ("p", id(dst), c)
            return [
                (base + k, (lambda kk: lambda: proj_unit(key, dst, w, x, c, kk))(k))
                for k in range(KT)
            ]

        def ofs(t, base):
            return [(base, lambda: outproj_1(t)),
                    (base + 1, lambda: outproj_2(t))]

        # pair-0 norms go LATE (slots 14/15): their results are consumed
        # whole sweeps later, and keeping projection units off slots 15/16
        # keeps the quarter-boundary PE window clear.
        cr = sweep(0, 1, carry_in=cr, fillers=[
            (14, nf(0, 0, 0)), (15, nf(0, 0, 1)),
            *pfs(qt[0], wq[0], xq, 2, 2), *pfs(qt[0], wq[0], xq, 3, 6),
            *pfs(kt[1], wk[1], xk, 0, 10),
        ])
        cr = sweep(0, 2, carry_in=cr, fillers=[
            (14, nf(0, 1, 0)), (15, nf(0, 1, 1)),
            *pfs(kt[1], wk[1], xk, 1, 2), *pfs(kt[1], wk[1], xk, 2, 6),
            *pfs(qt[1], wq[1], xq, 0, 10),
        ])
        cr = sweep(0, 3, carry_in=cr, fillers=[
            (14, nf(0, 2, 0)), (15, nf(0, 2, 1)),
            *pfs(kt[1], wk[1], xk, 3, 2), *pfs(qt[1], wq[1], xq, 1, 6),
            *pfs(qt[1], wq[1], xq, 2, 10),
        ])
        cr = sweep(1, 0, carry_in=cr, fillers=[
            (14, nf(0, 3, 0)), (15, nf(0, 3, 1)),
            *pfs(qt[1], wq[1], xq, 3, 2),
        ])
        cr = sweep(1, 1, carry_in=cr, fillers=[
            (3, nf(1, 0, 0)), (4, nf(1, 0, 1)),
            *ofs(0, 6), *ofs(1, 8), *ofs(2, 10), *ofs(3, 12),
        ])
        cr = sweep(1, 2, carry_in=cr, fillers=[
            (3, nf(1, 1, 0)), (4, nf(1, 1, 1)),
            *ofs(4, 6), *ofs(5, 8), *ofs(6, 10), *ofs(7, 12),
        ])
        cr = sweep(1, 3, carry_in=cr, fillers=[
            (3, nf(1, 2, 0)), (4, nf(1, 2, 1)),
            *ofs(8, 6), *ofs(9, 8), *ofs(10, 10), *ofs(11, 12),
        ])
        # tail: emit the last sweep's carried PV+drain, then normalize and
        # project out. (No hoisting: a held outproj PSUM tile plus the
        # norm's bc allocation deadlocks the 2-bank fill ring.)
        cr[0]()
        cr[1]()
        for s in range(2):
            norm_part(1, 3, s, tail=True)
        # tail outproj: the pair-0 matmul of each tile has no norm deps, so
        # emit them ahead (interleaved to keep the 2-bank fill ring
        # acyclic: each new ps alloc follows the copy that frees its bank)
        _tps = {}

        def tmm1(t):
            tsl = slice(t * 128, (t + 1) * 128)
            ps = fpp.tile([128, D], f32, tag="f", name="ops")
            _tps[t] = ps
            nc.tensor.matmul(ps[:], repP[0][:, tsl], wo[0][:],
                             start=True, stop=False)

        def tmm23(t):
            tsl = slice(t * 128, (t + 1) * 128)
            lsl = slice((t - 12) * 128, (t - 11) * 128)
            ps = _tps.pop(t)
            nc.tensor.matmul(ps[:], repP[1][0:E, tsl], wo[1][0:E, :],
                             start=False, stop=False)
            nc.tensor.matmul(ps[:], rsh[1][1][:, lsl], wo2[1][:],
                             start=False, stop=True)
            ost = ostp.tile([128, D], f32, tag="ost", name="ost")
            nc.vector.tensor_copy(ost[:], ps[:])
            nc.sync.dma_start(outp[t], ost[:])

        tmm1(12)
        tmm1(13)
        tmm23(12)
        tmm1(14)
        tmm23(13)
        tmm1(15)
        tmm23(14)
        tmm23(15)

    nc.compile()
    return nc


def _prep_core_inputs(c, x1, x2, v, Wq, Wk, Wv, Wo):
    bf = ml_dtypes.bfloat16
    b, g = c // 2, c % 2
    hs = slice(g * HPC, (g + 1) * HPC)
    wq = (Wq[hs] * (1.0 / np.sqrt(E))).astype(np.float32)   # fold 1/sqrt(E)
    wk, wv, wo = Wk[hs], Wv[hs], Wo[hs]

    def t_pack_pair(w):
        # [4,E,D] -> per pair p: concat(w[2p].T, w[2p+1].T, axis=1) [D,128]
        # -> k-subtile-major in the free dim: [2, 128, KT*128]
        out = np.empty((2, 128, KT * 128), bf)
        for p in range(2):
            m = np.concatenate([w[2 * p].T, w[2 * p + 1].T], axis=1)  # [D,128]
            out[p] = (m.reshape(KT, 128, 128).transpose(1, 0, 2)
                      .reshape(128, KT * 128).astype(bf))
        return out

    def x_chunks(x):
        # x[b].T [512, 2048] -> [chunk c, 128, KT*512]
        a = x.T.reshape(KT, 128, 4, 512).transpose(2, 1, 0, 3)
        return np.ascontiguousarray(a).astype(bf).reshape(4, 128, KT * 512)

    wvT = np.concatenate([wv[h].T for h in range(HPC)], axis=1)  # [D, 256]
    wvT = (wvT.reshape(KT, 128, HPC * E).transpose(1, 0, 2)
           .reshape(128, KT * HPC * E))
    # output weights packed in head pairs: [2, 2E=128, D]
    woP = np.stack([
        np.concatenate([wo[2 * p].T, wo[2 * p + 1].T], axis=0)
        for p in range(2)
    ])
    woS = np.stack([wo[2 * p + 1].T for p in range(2)])  # [2, E, D]
    return {
        "xqT": x_chunks(x2[b]), "xkT": x_chunks(x1[b]), "vT": x_chunks(v[b]),
        "wqT": t_pack_pair(wq), "wkT": t_pack_pair(wk),
        "wvT": np.ascontiguousarray(wvT).astype(bf),
        "woP": woP.astype(bf), "woS": woS.astype(bf),
    }


def kernel(**inputs):
    from concourse.bass_utils import run_bass_kernel_spmd

    x1 = np.asarray(inputs["x1"], np.float32)
    x2 = np.asarray(inputs["x2"], np.float32)
    v = np.asarray(inputs["v"], np.float32)
    Wq = np.asarray(inputs["Wq"], np.float32)
    Wk = np.asarray(inputs["Wk"], np.float32)
    Wv = np.asarray(inputs["Wv"], np.float32)
    Wo = np.asarray(inputs["Wo"], np.float32)

    if "nc" not in _CACHE:
        _CACHE["nc"] = _build()
    nc = _CACHE["nc"]

    in_maps = [
        _prep_core_inputs(c, x1, x2, v, Wq, Wk, Wv, Wo)
        for c in range(N_CORES)
    ]
    res = run_bass_kernel_spmd(nc, in_maps, list(range(N_CORES)))
    out = np.empty((B, N, D), np.float32)
    for b in range(B):
        out[b] = (
            res.results[2 * b]["outp"].reshape(N, D)
            + res.results[2 * b + 1]["outp"].reshape(N, D)
        )
    return out
